# revision 38
# baseline (speedup 1.0000x reference)
"""Trainium2 Bass kernel for nn_Attention (dense transformer attention w/ KV cache).

Math (matching the reference exactly):
  - Q/K/V projections of x; K/V appended to cache (returned).
  - Causal mask triu(ones(q, k_tot), 1) over k_tot=2048 masks every key index
    j > query index i.  Since queries occupy rows 0..1023 and the NEW keys
    occupy indices 1024..2047, attention output depends ONLY on the old cache
    (keys 0..q).  New K/V only appear in the returned cache.
  - Old-cache half of new_block_cache is an identity passthrough (host-side).

Sharding: 8 cores = 4 batches x 2 interleaved q-halves.
  half 0 -> q-blocks {7,4,3,0}, half 1 -> {6,5,2,1}  (blocks of 128 rows).
  Blocks ordered by descending causal extent so both halves run the SAME
  instruction stream with rounded extents [8,6,4,2]; mask differences are
  pure input data.

Device layout: scores computed transposed (k on partitions) so softmax-sum and
the pattern@V contraction need no transposes of the big pattern matrix; the
softmax denominator comes from a ones-column appended to V; normalization uses
a ones-matmul partition-broadcast of 1/denom.  x / K_old arrive pre-transposed
from the host (input layout prep).  Projections run in float32r (full-rate PE,
~1e-4 precision); attention internals in bf16.
"""

import sys

sys.path.insert(0, "/opt/trn_rl_repo")

import numpy as np  # noqa: E402

import concourse.bass as bass  # noqa: E402,F401
import concourse.tile as tile  # noqa: E402
from concourse import bacc, mybir  # noqa: E402
from concourse.bass_utils import run_bass_kernel_spmd  # noqa: E402

F32 = mybir.dt.float32
F32R = mybir.dt.float32r
BF16 = mybir.dt.bfloat16

B, H, D, M, Q, KV = 4, 12, 64, 768, 1024, 1024
P = 128
QC = 512  # q rows per core
NHP = 6  # head pairs
NKT = [512, 512, 384, 384, 256, 256, 128, 128]  # q-cols computed at key-tile kt
SLOTS = {0: [7, 4, 3, 0], 1: [6, 5, 2, 1]}  # q-block index per slot
TRUE_EXT = {0: [8, 5, 4, 1], 1: [7, 6, 3, 2]}  # true causal extent per slot

_program_cache = {}
LAST_RESULT = None


def build_program(bias_zero=True):
    key = ("nc", bias_zero)
    if key in _program_cache:
        return _program_cache[key]

    nc = bacc.Bacc(
        "TRN2",
        target_bir_lowering=False,
        debug=False,
        enable_asserts=True,
        num_devices=8,
    )

    xt_d = nc.dram_tensor("xt", [P, 6 * QC], F32R, kind="ExternalInput").ap()
    kt_d = nc.dram_tensor("kt", [P, NHP * KV], BF16, kind="ExternalInput").ap()
    va_d = nc.dram_tensor("v_aug", [P, 8 * H * 65], BF16, kind="ExternalInput").ap()
    wq_d = nc.dram_tensor("wq", [P, NHP * M], BF16, kind="ExternalInput").ap()
    xtb_d = nc.dram_tensor("xtb", [P, 6 * QC], BF16, kind="ExternalInput").ap()
    wk_d = nc.dram_tensor("wk", [P, 6 * H * D], F32R, kind="ExternalInput").ap()
    wv_d = nc.dram_tensor("wv", [P, 6 * H * D], F32R, kind="ExternalInput").ap()
    wo_d = nc.dram_tensor("wo", [P, 6 * M], F32R, kind="ExternalInput").ap()
    bq_d = nc.dram_tensor("bq", [P, NHP], F32, kind="ExternalInput").ap()
    bbc_d = nc.dram_tensor("bbc", [P, 3 * M], F32, kind="ExternalInput").ap()
    mask_d = nc.dram_tensor("masks", [P, 8 * 2 * P], BF16, kind="ExternalInput").ap()

    out_d = nc.dram_tensor("out", [QC, M], F32, kind="ExternalOutput").ap()
    kn_d = nc.dram_tensor("k_new", [QC, H * D], F32, kind="ExternalOutput").ap()
    vn_d = nc.dram_tensor("v_new", [QC, H * D], F32, kind="ExternalOutput").ap()

    with tile.TileContext(nc) as tc:
        _build_tile_kernel(
            tc, xt_d, xtb_d, kt_d, va_d, wq_d, wk_d, wv_d, wo_d, bq_d, bbc_d,
            mask_d, out_d, kn_d, vn_d, bias_zero=bias_zero,
        )

    nc.compile()
    _program_cache[key] = nc
    return nc


def _build_tile_kernel(
    tc, xt_d, xtb_d, kt_d, va_d, wq_d, wk_d, wv_d, wo_d, bq_d, bbc_d, mask_d,
    out_d, kn_d, vn_d, bias_zero=True,
):
    nc = tc.nc
    from contextlib import ExitStack

    with ExitStack() as ctx:
        persist = ctx.enter_context(tc.tile_pool(name="persist", bufs=1))
        pat_pool = ctx.enter_context(tc.tile_pool(name="pat", bufs=3))
        stage_pool = ctx.enter_context(tc.tile_pool(name="stage", bufs=3))
        recip_pool = ctx.enter_context(tc.tile_pool(name="recip", bufs=3))
        bcast_pool = ctx.enter_context(tc.tile_pool(name="bcast", bufs=3))
        # PSUM 8 banks: psA 2x[128,1024](2) + psZ 3x[65,512](1) + psK 1x[128,512](1)
        ps_a = ctx.enter_context(tc.tile_pool(name="psA", bufs=2, space="PSUM"))
        ps_z = ctx.enter_context(tc.tile_pool(name="psZ", bufs=3, space="PSUM"))
        ps_k = ctx.enter_context(tc.tile_pool(name="psK", bufs=1, space="PSUM"))

        def pt(shape, dtype, tag):
            return persist.tile(shape, dtype, tag=tag, name=tag)

        # ---- persistent SBUF tiles (packed; logical views are column slices) ----
        va_all = pt([P, 8 * H * 65], BF16, "vaall")
        m_all = pt([P, 8 * 2 * P], BF16, "maskall")
        wq_all = pt([P, NHP * M], BF16, "wqall")
        wk_all = pt([P, 6 * H * D], F32R, "wkall")
        wv_all = pt([P, 6 * H * D], F32R, "wvall")
        wo_all = pt([P, 6 * M], F32R, "woall")
        xT_all = pt([P, 6 * QC], F32R, "xTall")
        xTb_all = pt([P, 6 * QC], BF16, "xTball")
        kT_all = pt([P, NHP * KV], BF16, "kTall")
        va_sb = [va_all[:, i * H * 65 : (i + 1) * H * 65] for i in range(8)]
        m_sb = [m_all[:, i * 2 * P : (i + 1) * 2 * P] for i in range(8)]
        wq_sb = [wq_all[:, i * M : (i + 1) * M] for i in range(NHP)]
        wk_sb = [wk_all[:, i * H * D : (i + 1) * H * D] for i in range(6)]
        wv_sb = [wv_all[:, i * H * D : (i + 1) * H * D] for i in range(6)]
        wo_sb = [wo_all[:, i * M : (i + 1) * M] for i in range(6)]
        xT = [xT_all[:, i * QC : (i + 1) * QC] for i in range(6)]
        xTb = [xTb_all[:, i * QC : (i + 1) * QC] for i in range(6)]
        kT = [kT_all[:, i * KV : (i + 1) * KV] for i in range(NHP)]
        qt = [pt([P, QC], BF16, f"qt{i}") for i in range(NHP)]
        znorm = [pt([P, QC], F32R, f"zn{i}") for i in range(NHP)]
        ones_b = pt([1, P], BF16, "onesb")
        bq_sb = pt([P, NHP], F32, "bq")
        bbc_all = pt([P, 3 * M], F32, "bbcall")
        bias_bc = [bbc_all[:, i * M : (i + 1) * M] for i in range(3)]

        # ---- input DMAs: QT-critical stream on the scalar HWDGE ring,
        # everything else FIFO on the sync ring (fine-grained arrival) ----
        nc.gpsimd.memset(ones_b[:], 1.0)
        nc.sync.dma_start(bq_sb[:], bq_d[:])
        for i in range(6):
            nc.sync.dma_start(xTb[i], xtb_d[:, i * QC : (i + 1) * QC])
        for i in range(NHP):
            nc.sync.dma_start(wq_sb[i], wq_d[:, i * M : (i + 1) * M])
        for i in range(NHP):
            nc.sync.dma_start(kT[i], kt_d[:, i * KV : (i + 1) * KV])
        for i in range(8):
            nc.sync.dma_start(va_sb[i], va_d[:, i * H * 65 : (i + 1) * H * 65])
            nc.sync.dma_start(m_sb[i], mask_d[:, i * 2 * P : (i + 1) * 2 * P])
        for i in range(6):
            nc.sync.dma_start(xT[i], xt_d[:, i * QC : (i + 1) * QC])
        for i in range(6):
            nc.sync.dma_start(wk_sb[i], wk_d[:, i * H * D : (i + 1) * H * D])
            nc.sync.dma_start(wv_sb[i], wv_d[:, i * H * D : (i + 1) * H * D])
        nc.sync.dma_start(bbc_all[:], bbc_d[:])
        for i in range(6):
            nc.sync.dma_start(wo_sb[i], wo_d[:, i * M : (i + 1) * M])

        # ---- PE warm-up so HAM unthrottles during the DMA front ----
        for wi in range(16):
            wp = ps_k.tile([P, P], F32, tag="psk", name=f"warm{wi}")
            nc.tensor.matmul(wp[:], ones_b[:], ones_b[:])

        # ---- QT projection: [d(2 heads), q] per head pair ----
        for hp in range(NHP):
            qp = ps_a.tile([P, QC], F32, tag="psa", name=f"pqt{hp}")
            for kt in range(6):
                nc.tensor.matmul(
                    qp[:],
                    wq_sb[hp][:, kt * P : (kt + 1) * P],
                    xTb[kt][:],
                    start=(kt == 0),
                    stop=(kt == 5),
                )
            nc.scalar.activation(
                qt[hp][:],
                qp[:],
                mybir.ActivationFunctionType.Identity,
                bias=bq_sb[:, hp : hp + 1],
            )

        # ---- K/V_new half-group fillers (hide in exp-gated PE bubbles) ----
        fillers = []
        kv_stages = {}
        for s in range(4):
            for w_sb, dst in ((wk_sb, kn_d), (wv_sb, vn_d)):
                ksb = stage_pool.tile(
                    [P, M], F32, tag="stage", name=f"skv{s}_{dst.name}", bufs=3
                )
                kv_stages[(s, dst.name)] = ksb
                for n0, n1 in ((0, 512), (512, 768)):
                    fillers.append(("kv", s, w_sb, dst, ksb, n0, n1))

        _fill_ctr = [0]

        def emit_filler(f):
            _, s, w_sb, dst, ksb, n0, n1 = f
            _fill_ctr[0] += 1
            if _fill_ctr[0] % 2 == 0:
                kp = ps_k.tile(
                    [P, n1 - n0], F32, tag="psk", name=f"pk{s}_{dst.name}_{n0}"
                )
            else:
                kp = ps_z.tile(
                    [P, n1 - n0], F32, tag="psz", name=f"pk{s}_{dst.name}_{n0}"
                )
            for kt6 in range(6):
                nc.tensor.matmul(
                    kp[:],
                    xT[kt6][:, s * P : (s + 1) * P],
                    w_sb[kt6][:, n0:n1],
                    start=(kt6 == 0),
                    stop=(kt6 == 5),
                )
            bb = bias_bc[0] if dst is kn_d else bias_bc[1]
            if bias_zero:
                nc.vector.tensor_copy(ksb[:, n0:n1], kp[:])
            else:
                nc.vector.tensor_add(ksb[:, n0:n1], kp[:], bb[:, n0:n1])
            if n1 == M:
                nc.sync.dma_start(dst[s * P : (s + 1) * P, :], ksb[:])

        # ---- attention: software-pipelined stages ----
        # stage st emits scores/exp/mask for hp=st and z-matmuls for hp=st-1
        zps_all = {}
        pats_all = {}
        fill_iter = iter(fillers)

        def scores_stage(hp):
            pats = []
            for kt in range(8):
                n = NKT[kt]
                sp = ps_a.tile([P, 1024], F32, tag="psa", name=f"ps{hp}_{kt}")
                pat = pat_pool.tile(
                    [P, 1024], BF16, tag=f"pat{kt}", name=f"pat{hp}_{kt}", bufs=2
                )
                pats.append(pat)
                for hh in range(2):
                    nc.tensor.matmul(
                        sp[:, hh * 512 : hh * 512 + n],
                        kT[hp][hh * D : (hh + 1) * D, kt * P : (kt + 1) * P],
                        qt[hp][hh * D : (hh + 1) * D, 0:n],
                    )
                spv = sp[:].rearrange("p (h n2) -> p h n2", h=2)[:, :, 0:n]
                patv = pat[:].rearrange("p (h n2) -> p h n2", h=2)[:, :, 0:n]
                nc.scalar.activation(
                    patv,
                    spv,
                    mybir.ActivationFunctionType.Exp,
                    scale=0.125,
                )
                pv = pat[:].rearrange("p (h n) -> p h n", h=2)[:, :, n - P : n]
                mv = m_sb[kt][:].rearrange("p (h n) -> p h n", h=2)
                nc.gpsimd.tensor_mul(pv, pv, mv)
                yield None
            pats_all[hp] = pats

        def z_stage(hp):
            zps = [
                ps_z.tile([65, QC], F32, tag="psz", name=f"pz{hp}_{_h}")
                for _h in range(2)
            ]
            zps_all[hp] = zps
            pats = pats_all[hp]
            for kt in range(8):
                n = NKT[kt]
                for hh in range(2):
                    h = 2 * hp + hh
                    nc.tensor.matmul(
                        zps[hh][:, 0:n],
                        va_sb[kt][:, h * 65 : (h + 1) * 65],
                        pats[kt][:, hh * 512 : hh * 512 + n],
                        start=(kt == 0),
                        stop=(kt == 7),
                    )
                yield None

        def norm_stage(hp):
            zps = zps_all[hp]
            for hh in range(2):
                zsb = bcast_pool.tile([D, QC], F32, tag="zsb", name=f"zs{hp}_{hh}")
                nc.vector.tensor_copy(zsb[:], zps[hh][0:D, :])
                den = recip_pool.tile([1, QC], F32, tag="den", name=f"dn{hp}_{hh}")
                nc.scalar.copy(den[:], zps[hh][D : D + 1, :])
                rc = recip_pool.tile([1, QC], F32, tag="recip", name=f"rc{hp}_{hh}")
                rcb = recip_pool.tile([1, QC], BF16, tag="rcb", name=f"rb{hp}_{hh}")
                nc.vector.reciprocal_approx_fast(rc[:], den[:])
                nc.vector.tensor_copy(rcb[:], rc[:])  # cast f32 -> bf16
                bc = ps_z.tile([D, QC], F32, tag="psz", name=f"pbc{hp}_{hh}")
                nc.tensor.matmul(bc[:], ones_b[0:1, 0:D], rcb[:])
                with nc.allow_low_precision(reason="f32r stores full fp32 bits"):
                    nc.vector.tensor_mul(
                        znorm[hp][hh * D : (hh + 1) * D, :], zsb[:], bc[:]
                    )

        for st in range(NHP + 1):
            sgen = scores_stage(st) if st < NHP else None
            zgen = z_stage(st - 1) if st >= 1 else None
            for kt in range(8):
                if sgen is not None:
                    next(sgen, None)
                if zgen is not None:
                    next(zgen, None)
            if sgen is not None:
                for _ in sgen:
                    pass
            if zgen is not None:
                for _ in zgen:
                    pass
            if st >= 1:
                norm_stage(st - 1)
            # K/V_new fillers: ~2-3 half-groups per stage
            for _ in range(3 if st >= 1 else 0):
                f = next(fill_iter, None)
                if f is not None:
                    emit_filler(f)
        for f in fill_iter:
            emit_filler(f)

        # ---- output projection (split-bank accumulation) ----
        for s in range(4):
            osb = stage_pool.tile([P, M], F32, tag="ostage", name=f"so{s}", bufs=2)
            for n0, n1 in ((0, 512), (512, 768)):
                opool = ps_k if s % 2 == 0 else ps_a
                op = opool.tile(
                    [P, n1 - n0], F32, tag="psk" if s % 2 == 0 else "psa",
                    name=f"po{s}_{n0}",
                )
                for hp in range(NHP):
                    nc.tensor.matmul(
                        op[:],
                        znorm[hp][:, s * P : (s + 1) * P],
                        wo_sb[hp][:, n0:n1],
                        start=(hp == 0),
                        stop=(hp == NHP - 1),
                    )
                if bias_zero:
                    nc.vector.tensor_copy(osb[:, n0:n1], op[:])
                else:
                    nc.vector.tensor_add(osb[:, n0:n1], op[:], bias_bc[2][:, n0:n1])
            nc.sync.dma_start(out_d[s * P : (s + 1) * P, :], osb[:])


def _make_masks(half):
    import ml_dtypes

    tri = np.triu(np.ones((P, P), ml_dtypes.bfloat16))  # [ki, qi] valid iff ki <= qi
    ones = np.ones((P, P), ml_dtypes.bfloat16)
    zero = np.zeros((P, P), ml_dtypes.bfloat16)
    types = {
        0: [tri, zero, ones, tri, tri, zero, ones, tri],
        1: [ones, tri, tri, zero, ones, tri, tri, zero],
    }[half]
    return np.stack([np.concatenate([t, t], axis=1) for t in types])  # [8,128,256]


def _host_inputs(inputs):
    """Core-invariant rearranged tensors."""
    WQ = np.asarray(inputs["W_Q"], np.float32)
    WK = np.asarray(inputs["W_K"], np.float32)
    WV = np.asarray(inputs["W_V"], np.float32)
    WO = np.asarray(inputs["W_O"], np.float32)
    bQ = np.asarray(inputs["b_Q"], np.float32)
    bK = np.asarray(inputs["b_K"], np.float32)
    bV = np.asarray(inputs["b_V"], np.float32)
    bO = np.asarray(inputs["b_O"], np.float32)

    import ml_dtypes

    def pack(a):  # [G, P, W] -> [P, G*W]
        return np.ascontiguousarray(a.transpose(1, 0, 2).reshape(P, -1))

    # wq: [hp, p, kt*128 + hh*64 + dd] = WQ[2hp+hh, kt*128+p, dd]
    w = WQ.reshape(NHP, 2, 6, P, D)  # [hp, hh, kt, p, d]
    wq_r = pack(
        w.transpose(0, 3, 2, 1, 4).reshape(NHP, P, M).astype(ml_dtypes.bfloat16)
    )
    # wk/wv: [kt, p, h*64+dd] = W[h, kt*128+p, dd]
    wk_r = pack(WK.reshape(H, 6, P, D).transpose(1, 2, 0, 3).reshape(6, P, H * D))
    wv_r = pack(WV.reshape(H, 6, P, D).transpose(1, 2, 0, 3).reshape(6, P, H * D))
    wo_r = pack(WO.reshape(H * D, M).reshape(6, P, M))
    bq_r = np.ascontiguousarray(bQ.reshape(NHP, P).T)  # [128, 6]
    bbc = pack(
        np.stack(
            [
                np.tile(bK.reshape(1, -1), (P, 1)),
                np.tile(bV.reshape(1, -1), (P, 1)),
                np.tile(bO.reshape(1, -1), (P, 1)),
            ]
        ).astype(np.float32)
    )
    return dict(wq=wq_r, wk=wk_r, wv=wv_r, wo=wo_r, bq=bq_r, bbc=bbc)


def _core_inputs(inputs, shared, core):
    x = np.asarray(inputs["normalized_resid_pre"], np.float32)
    cache = np.asarray(inputs["block_cache"], np.float32)
    bi, half = core // 2, core % 2
    x_c = np.concatenate([x[bi, J * P : (J + 1) * P, :] for J in SLOTS[half]], axis=0)
    xt = x_c.T.reshape(6, P, QC)
    import ml_dtypes

    def pack(a):  # [G, P, W] -> [P, G*W]
        return np.ascontiguousarray(a.transpose(1, 0, 2).reshape(P, -1))

    ko = cache[bi, 0]  # [1024, h, d]
    ktr = pack(ko.transpose(1, 2, 0).reshape(NHP, P, KV).astype(ml_dtypes.bfloat16))
    v = cache[bi, 1].reshape(8, P, H, D)
    va = np.ones((8, P, H, 65), ml_dtypes.bfloat16)
    va[..., :D] = v.astype(ml_dtypes.bfloat16)
    return dict(
        xt=pack(xt),
        xtb=pack(xt.astype(ml_dtypes.bfloat16)),
        kt=ktr,
        v_aug=pack(va.reshape(8, P, H * 65)),
        masks=pack(_make_masks(half).astype(ml_dtypes.bfloat16)),
        **shared,
    )


def kernel(**inputs):
    bias_zero = all(
        not np.any(np.asarray(inputs[k])) for k in ("b_K", "b_V", "b_O")
    )
    nc = build_program(bias_zero=bias_zero)
    shared = _host_inputs(inputs)
    in_maps = [_core_inputs(inputs, shared, c) for c in range(8)]
    res = run_bass_kernel_spmd(nc, in_maps, core_ids=list(range(8)))
    global LAST_RESULT
    LAST_RESULT = res

    cache = np.asarray(inputs["block_cache"], np.float32)
    out = np.empty((B, Q, M), np.float32)
    new_cache = np.empty((B, 2, 2 * KV, H, D), np.float32)
    new_cache[:, :, :KV] = cache
    for c in range(8):
        bi, half = c // 2, c % 2
        r = res.results[c]
        for si, J in enumerate(SLOTS[half]):
            rows = slice(si * P, (si + 1) * P)
            out[bi, J * P : (J + 1) * P] = r["out"][rows]
            new_cache[bi, 0, KV + J * P : KV + (J + 1) * P] = r["k_new"][rows].reshape(
                P, H, D
            )
            new_cache[bi, 1, KV + J * P : KV + (J + 1) * P] = r["v_new"][rows].reshape(
                P, H, D
            )
    return out, new_cache


# revision 39
# speedup vs baseline: 1.0712x; 1.0712x over previous
"""Trainium2 Bass kernel for nn_Attention (dense transformer attention w/ KV cache).

Math (matching the reference exactly):
  - Q/K/V projections of x; K/V appended to cache (returned).
  - Causal mask triu(ones(q, k_tot), 1) over k_tot=2048 masks every key index
    j > query index i.  Since queries occupy rows 0..1023 and the NEW keys
    occupy indices 1024..2047, attention output depends ONLY on the old cache
    (keys 0..q).  New K/V only appear in the returned cache.
  - Old-cache half of new_block_cache is an identity passthrough (host-side).

Sharding: 8 cores = 4 batches x 2 interleaved q-halves.
  half 0 -> q-blocks {7,4,3,0}, half 1 -> {6,5,2,1}  (blocks of 128 rows).
  Blocks ordered by descending causal extent so both halves run the SAME
  instruction stream with rounded extents [8,6,4,2]; mask differences are
  pure input data.

Device layout: scores computed transposed (k on partitions) so softmax-sum and
the pattern@V contraction need no transposes of the big pattern matrix; the
softmax denominator comes from a ones-column appended to V; normalization uses
a ones-matmul partition-broadcast of 1/denom.  x / K_old arrive pre-transposed
from the host (input layout prep).  Projections run in float32r (full-rate PE,
~1e-4 precision); attention internals in bf16.
"""

import sys

sys.path.insert(0, "/opt/trn_rl_repo")

import numpy as np  # noqa: E402

import concourse.bass as bass  # noqa: E402,F401
import concourse.tile as tile  # noqa: E402
from concourse import bacc, mybir  # noqa: E402
from concourse.bass_utils import run_bass_kernel_spmd  # noqa: E402

F32 = mybir.dt.float32
F32R = mybir.dt.float32r
BF16 = mybir.dt.bfloat16

B, H, D, M, Q, KV = 4, 12, 64, 768, 1024, 1024
P = 128
QC = 512  # q rows per core
NHP = 6  # head pairs
NKT = [512, 512, 384, 384, 256, 256, 128, 128]  # q-cols computed at key-tile kt
SLOTS = {0: [7, 4, 3, 0], 1: [6, 5, 2, 1]}  # q-block index per slot
TRUE_EXT = {0: [8, 5, 4, 1], 1: [7, 6, 3, 2]}  # true causal extent per slot

_program_cache = {}
LAST_RESULT = None


def build_program(bias_zero=True):
    key = ("nc", bias_zero)
    if key in _program_cache:
        return _program_cache[key]

    nc = bacc.Bacc(
        "TRN2",
        target_bir_lowering=False,
        debug=False,
        enable_asserts=True,
        num_devices=8,
    )

    xt_d = nc.dram_tensor("xt", [P, 6 * QC], F32R, kind="ExternalInput").ap()
    kt_d = nc.dram_tensor("kt", [P, NHP * KV], BF16, kind="ExternalInput").ap()
    va_d = nc.dram_tensor("v_aug", [P, 8 * H * 65], BF16, kind="ExternalInput").ap()
    wq_d = nc.dram_tensor("wq", [P, NHP * M], BF16, kind="ExternalInput").ap()
    xtb_d = nc.dram_tensor("xtb", [P, 6 * QC], BF16, kind="ExternalInput").ap()
    wk_d = nc.dram_tensor("wk", [P, 6 * H * D], F32R, kind="ExternalInput").ap()
    wv_d = nc.dram_tensor("wv", [P, 6 * H * D], F32R, kind="ExternalInput").ap()
    wo_d = nc.dram_tensor("wo", [P, 6 * M], F32R, kind="ExternalInput").ap()
    bq_d = nc.dram_tensor("bq", [P, NHP], F32, kind="ExternalInput").ap()
    bbc_d = nc.dram_tensor("bbc", [P, 3 * M], F32, kind="ExternalInput").ap()
    mask_d = nc.dram_tensor("masks", [P, 8 * 2 * P], BF16, kind="ExternalInput").ap()

    out_d = nc.dram_tensor("out", [QC, M], F32, kind="ExternalOutput").ap()
    kn_d = nc.dram_tensor("k_new", [QC, H * D], F32, kind="ExternalOutput").ap()
    vn_d = nc.dram_tensor("v_new", [QC, H * D], F32, kind="ExternalOutput").ap()

    with tile.TileContext(nc) as tc:
        _build_tile_kernel(
            tc, xt_d, xtb_d, kt_d, va_d, wq_d, wk_d, wv_d, wo_d, bq_d, bbc_d,
            mask_d, out_d, kn_d, vn_d, bias_zero=bias_zero,
        )

    nc.compile()
    _program_cache[key] = nc
    return nc


def _build_tile_kernel(
    tc, xt_d, xtb_d, kt_d, va_d, wq_d, wk_d, wv_d, wo_d, bq_d, bbc_d, mask_d,
    out_d, kn_d, vn_d, bias_zero=True,
):
    nc = tc.nc
    from contextlib import ExitStack

    with ExitStack() as ctx:
        persist = ctx.enter_context(tc.tile_pool(name="persist", bufs=1))
        pat_pool = ctx.enter_context(tc.tile_pool(name="pat", bufs=3))
        stage_pool = ctx.enter_context(tc.tile_pool(name="stage", bufs=3))
        recip_pool = ctx.enter_context(tc.tile_pool(name="recip", bufs=3))
        bcast_pool = ctx.enter_context(tc.tile_pool(name="bcast", bufs=3))
        # PSUM 8 banks: psA 2x[128,1024](2) + psZ 3x[65,512](1) + psK 1x[128,512](1)
        ps_a = ctx.enter_context(tc.tile_pool(name="psA", bufs=2, space="PSUM"))
        ps_z = ctx.enter_context(tc.tile_pool(name="psZ", bufs=3, space="PSUM"))
        ps_k = ctx.enter_context(tc.tile_pool(name="psK", bufs=1, space="PSUM"))

        def pt(shape, dtype, tag):
            return persist.tile(shape, dtype, tag=tag, name=tag)

        # ---- persistent SBUF tiles (packed; logical views are column slices) ----
        va_all = pt([P, 8 * H * 65], BF16, "vaall")
        m_all = pt([P, 8 * 2 * P], BF16, "maskall")
        wq_all = pt([P, NHP * M], BF16, "wqall")
        wk_all = pt([P, 6 * H * D], F32R, "wkall")
        wv_all = pt([P, 6 * H * D], F32R, "wvall")
        wo_all = pt([P, 6 * M], F32R, "woall")
        xT_all = pt([P, 6 * QC], F32R, "xTall")
        xTb_all = pt([P, 6 * QC], BF16, "xTball")
        kT_all = pt([P, NHP * KV], BF16, "kTall")
        va_sb = [va_all[:, i * H * 65 : (i + 1) * H * 65] for i in range(8)]
        m_sb = [m_all[:, i * 2 * P : (i + 1) * 2 * P] for i in range(8)]
        wq_sb = [wq_all[:, i * M : (i + 1) * M] for i in range(NHP)]
        wk_sb = [wk_all[:, i * H * D : (i + 1) * H * D] for i in range(6)]
        wv_sb = [wv_all[:, i * H * D : (i + 1) * H * D] for i in range(6)]
        wo_sb = [wo_all[:, i * M : (i + 1) * M] for i in range(6)]
        xT = [xT_all[:, i * QC : (i + 1) * QC] for i in range(6)]
        xTb = [xTb_all[:, i * QC : (i + 1) * QC] for i in range(6)]
        kT = [kT_all[:, i * KV : (i + 1) * KV] for i in range(NHP)]
        qt = [pt([P, QC], BF16, f"qt{i}") for i in range(NHP)]
        znorm = [pt([P, QC], F32R, f"zn{i}") for i in range(NHP)]
        ones_b = pt([1, P], BF16, "onesb")
        bq_sb = pt([P, NHP], F32, "bq")
        bbc_all = pt([P, 3 * M], F32, "bbcall")
        bias_bc = [bbc_all[:, i * M : (i + 1) * M] for i in range(3)]

        # ---- input DMAs: QT-critical stream on the scalar HWDGE ring,
        # everything else FIFO on the sync ring (fine-grained arrival) ----
        nc.gpsimd.memset(ones_b[:], 1.0)
        nc.sync.dma_start(bq_sb[:], bq_d[:])
        for i in range(6):
            nc.sync.dma_start(xTb[i], xtb_d[:, i * QC : (i + 1) * QC])
        for i in range(NHP):
            nc.sync.dma_start(wq_sb[i], wq_d[:, i * M : (i + 1) * M])
        for i in range(NHP):
            nc.sync.dma_start(kT[i], kt_d[:, i * KV : (i + 1) * KV])
        for i in range(8):
            nc.sync.dma_start(va_sb[i], va_d[:, i * H * 65 : (i + 1) * H * 65])
            nc.sync.dma_start(m_sb[i], mask_d[:, i * 2 * P : (i + 1) * 2 * P])
        for i in range(6):
            nc.sync.dma_start(xT[i], xt_d[:, i * QC : (i + 1) * QC])
        for i in range(6):
            nc.sync.dma_start(wk_sb[i], wk_d[:, i * H * D : (i + 1) * H * D])
            nc.sync.dma_start(wv_sb[i], wv_d[:, i * H * D : (i + 1) * H * D])
        nc.sync.dma_start(bbc_all[:], bbc_d[:])
        for i in range(6):
            nc.sync.dma_start(wo_sb[i], wo_d[:, i * M : (i + 1) * M])

        # ---- PE warm-up so HAM unthrottles during the DMA front ----
        for wi in range(16):
            wp = ps_k.tile([P, P], F32, tag="psk", name=f"warm{wi}")
            nc.tensor.matmul(wp[:], ones_b[:], ones_b[:])

        # ---- QT projection: [d(2 heads), q] per head pair ----
        for hp in range(NHP):
            qp = ps_a.tile([P, QC], F32, tag="psa", name=f"pqt{hp}")
            for kt in range(6):
                nc.tensor.matmul(
                    qp[:],
                    wq_sb[hp][:, kt * P : (kt + 1) * P],
                    xTb[kt][:],
                    start=(kt == 0),
                    stop=(kt == 5),
                )
            nc.scalar.activation(
                qt[hp][:],
                qp[:],
                mybir.ActivationFunctionType.Identity,
                bias=bq_sb[:, hp : hp + 1],
            )

        # ---- K/V_new half-group fillers (hide in exp-gated PE bubbles) ----
        fillers = []
        kv_stages = {}
        for s in range(4):
            for w_sb, dst in ((wk_sb, kn_d), (wv_sb, vn_d)):
                ksb = stage_pool.tile(
                    [P, M], F32, tag="stage", name=f"skv{s}_{dst.name}", bufs=3
                )
                kv_stages[(s, dst.name)] = ksb
                for n0, n1 in ((0, 512), (512, 768)):
                    fillers.append(("kv", s, w_sb, dst, ksb, n0, n1))

        def emit_filler(f):
            _, s, w_sb, dst, ksb, n0, n1 = f
            kp = ps_k.tile([P, n1 - n0], F32, tag="psk", name=f"pk{s}_{dst.name}_{n0}")
            for kt6 in range(6):
                nc.tensor.matmul(
                    kp[:],
                    xT[kt6][:, s * P : (s + 1) * P],
                    w_sb[kt6][:, n0:n1],
                    start=(kt6 == 0),
                    stop=(kt6 == 5),
                )
            bb = bias_bc[0] if dst is kn_d else bias_bc[1]
            if bias_zero:
                nc.vector.tensor_copy(ksb[:, n0:n1], kp[:])
            else:
                nc.vector.tensor_add(ksb[:, n0:n1], kp[:], bb[:, n0:n1])
            if n1 == M:
                nc.sync.dma_start(dst[s * P : (s + 1) * P, :], ksb[:])

        # ---- attention: software-pipelined stages ----
        # stage st emits scores/exp/mask for hp=st and z-matmuls for hp=st-1
        zps_all = {}
        pats_all = {}
        fill_iter = iter(fillers)

        def scores_stage(hp):
            pats = []
            for kt in range(8):
                n = NKT[kt]
                sp = ps_a.tile([P, 1024], F32, tag="psa", name=f"ps{hp}_{kt}")
                pat = pat_pool.tile(
                    [P, 1024], BF16, tag=f"pat{kt}", name=f"pat{hp}_{kt}", bufs=2
                )
                pats.append(pat)
                for hh in range(2):
                    nc.tensor.matmul(
                        sp[:, hh * 512 : hh * 512 + n],
                        kT[hp][hh * D : (hh + 1) * D, kt * P : (kt + 1) * P],
                        qt[hp][hh * D : (hh + 1) * D, 0:n],
                    )
                spv = sp[:].rearrange("p (h n2) -> p h n2", h=2)[:, :, 0:n]
                patv = pat[:].rearrange("p (h n2) -> p h n2", h=2)[:, :, 0:n]
                nc.scalar.activation(
                    patv,
                    spv,
                    mybir.ActivationFunctionType.Exp,
                    scale=0.125,
                )
                pv = pat[:].rearrange("p (h n) -> p h n", h=2)[:, :, n - P : n]
                mv = m_sb[kt][:].rearrange("p (h n) -> p h n", h=2)
                nc.gpsimd.tensor_mul(pv, pv, mv)
                yield None
            pats_all[hp] = pats

        def z_stage(hp):
            zps = [
                ps_z.tile([65, QC], F32, tag="psz", name=f"pz{hp}_{_h}")
                for _h in range(2)
            ]
            zps_all[hp] = zps
            pats = pats_all[hp]
            for kt in range(8):
                n = NKT[kt]
                for hh in range(2):
                    h = 2 * hp + hh
                    nc.tensor.matmul(
                        zps[hh][:, 0:n],
                        va_sb[kt][:, h * 65 : (h + 1) * 65],
                        pats[kt][:, hh * 512 : hh * 512 + n],
                        start=(kt == 0),
                        stop=(kt == 7),
                    )
                yield None

        def norm_stage(hp):
            zps = zps_all[hp]
            for hh in range(2):
                zsb = bcast_pool.tile([D, QC], F32, tag="zsb", name=f"zs{hp}_{hh}")
                nc.vector.tensor_copy(zsb[:], zps[hh][0:D, :])
                den = recip_pool.tile([1, QC], F32, tag="den", name=f"dn{hp}_{hh}")
                nc.scalar.copy(den[:], zps[hh][D : D + 1, :])
                rc = recip_pool.tile([1, QC], F32, tag="recip", name=f"rc{hp}_{hh}")
                rcb = recip_pool.tile([1, QC], BF16, tag="rcb", name=f"rb{hp}_{hh}")
                nc.vector.reciprocal_approx_fast(rc[:], den[:])
                nc.vector.tensor_copy(rcb[:], rc[:])  # cast f32 -> bf16
                bc = ps_z.tile([D, QC], F32, tag="psz", name=f"pbc{hp}_{hh}")
                nc.tensor.matmul(bc[:], ones_b[0:1, 0:D], rcb[:])
                with nc.allow_low_precision(reason="f32r stores full fp32 bits"):
                    nc.vector.tensor_mul(
                        znorm[hp][hh * D : (hh + 1) * D, :], zsb[:], bc[:]
                    )

        for st in range(NHP + 1):
            sgen = scores_stage(st) if st < NHP else None
            zgen = z_stage(st - 1) if st >= 1 else None
            for kt in range(8):
                if sgen is not None:
                    next(sgen, None)
                if zgen is not None:
                    next(zgen, None)
            if sgen is not None:
                for _ in sgen:
                    pass
            if zgen is not None:
                for _ in zgen:
                    pass
            if st >= 1:
                norm_stage(st - 1)
            # K/V_new fillers: ~2-3 half-groups per stage
            for _ in range(3 if st >= 1 else 0):
                f = next(fill_iter, None)
                if f is not None:
                    emit_filler(f)
        for f in fill_iter:
            emit_filler(f)

        # ---- output projection (split-bank accumulation) ----
        for s in range(4):
            osb = stage_pool.tile([P, M], F32, tag="ostage", name=f"so{s}", bufs=2)
            for n0, n1 in ((0, 512), (512, 768)):
                opool = ps_k if s % 2 == 0 else ps_a
                op = opool.tile(
                    [P, n1 - n0], F32, tag="psk" if s % 2 == 0 else "psa",
                    name=f"po{s}_{n0}",
                )
                for hp in range(NHP):
                    nc.tensor.matmul(
                        op[:],
                        znorm[hp][:, s * P : (s + 1) * P],
                        wo_sb[hp][:, n0:n1],
                        start=(hp == 0),
                        stop=(hp == NHP - 1),
                    )
                if bias_zero:
                    nc.vector.tensor_copy(osb[:, n0:n1], op[:])
                else:
                    nc.vector.tensor_add(osb[:, n0:n1], op[:], bias_bc[2][:, n0:n1])
                nc.sync.dma_start(out_d[s * P : (s + 1) * P, n0:n1], osb[:, n0:n1])


def _make_masks(half):
    import ml_dtypes

    tri = np.triu(np.ones((P, P), ml_dtypes.bfloat16))  # [ki, qi] valid iff ki <= qi
    ones = np.ones((P, P), ml_dtypes.bfloat16)
    zero = np.zeros((P, P), ml_dtypes.bfloat16)
    types = {
        0: [tri, zero, ones, tri, tri, zero, ones, tri],
        1: [ones, tri, tri, zero, ones, tri, tri, zero],
    }[half]
    return np.stack([np.concatenate([t, t], axis=1) for t in types])  # [8,128,256]


def _host_inputs(inputs):
    """Core-invariant rearranged tensors."""
    WQ = np.asarray(inputs["W_Q"], np.float32)
    WK = np.asarray(inputs["W_K"], np.float32)
    WV = np.asarray(inputs["W_V"], np.float32)
    WO = np.asarray(inputs["W_O"], np.float32)
    bQ = np.asarray(inputs["b_Q"], np.float32)
    bK = np.asarray(inputs["b_K"], np.float32)
    bV = np.asarray(inputs["b_V"], np.float32)
    bO = np.asarray(inputs["b_O"], np.float32)

    import ml_dtypes

    def pack(a):  # [G, P, W] -> [P, G*W]
        return np.ascontiguousarray(a.transpose(1, 0, 2).reshape(P, -1))

    # wq: [hp, p, kt*128 + hh*64 + dd] = WQ[2hp+hh, kt*128+p, dd]
    w = WQ.reshape(NHP, 2, 6, P, D)  # [hp, hh, kt, p, d]
    wq_r = pack(
        w.transpose(0, 3, 2, 1, 4).reshape(NHP, P, M).astype(ml_dtypes.bfloat16)
    )
    # wk/wv: [kt, p, h*64+dd] = W[h, kt*128+p, dd]
    wk_r = pack(WK.reshape(H, 6, P, D).transpose(1, 2, 0, 3).reshape(6, P, H * D))
    wv_r = pack(WV.reshape(H, 6, P, D).transpose(1, 2, 0, 3).reshape(6, P, H * D))
    wo_r = pack(WO.reshape(H * D, M).reshape(6, P, M))
    bq_r = np.ascontiguousarray(bQ.reshape(NHP, P).T)  # [128, 6]
    bbc = pack(
        np.stack(
            [
                np.tile(bK.reshape(1, -1), (P, 1)),
                np.tile(bV.reshape(1, -1), (P, 1)),
                np.tile(bO.reshape(1, -1), (P, 1)),
            ]
        ).astype(np.float32)
    )
    return dict(wq=wq_r, wk=wk_r, wv=wv_r, wo=wo_r, bq=bq_r, bbc=bbc)


def _core_inputs(inputs, shared, core):
    x = np.asarray(inputs["normalized_resid_pre"], np.float32)
    cache = np.asarray(inputs["block_cache"], np.float32)
    bi, half = core // 2, core % 2
    x_c = np.concatenate([x[bi, J * P : (J + 1) * P, :] for J in SLOTS[half]], axis=0)
    xt = x_c.T.reshape(6, P, QC)
    import ml_dtypes

    def pack(a):  # [G, P, W] -> [P, G*W]
        return np.ascontiguousarray(a.transpose(1, 0, 2).reshape(P, -1))

    ko = cache[bi, 0]  # [1024, h, d]
    ktr = pack(ko.transpose(1, 2, 0).reshape(NHP, P, KV).astype(ml_dtypes.bfloat16))
    v = cache[bi, 1].reshape(8, P, H, D)
    va = np.ones((8, P, H, 65), ml_dtypes.bfloat16)
    va[..., :D] = v.astype(ml_dtypes.bfloat16)
    return dict(
        xt=pack(xt),
        xtb=pack(xt.astype(ml_dtypes.bfloat16)),
        kt=ktr,
        v_aug=pack(va.reshape(8, P, H * 65)),
        masks=pack(_make_masks(half).astype(ml_dtypes.bfloat16)),
        **shared,
    )


def kernel(**inputs):
    bias_zero = all(
        not np.any(np.asarray(inputs[k])) for k in ("b_K", "b_V", "b_O")
    )
    nc = build_program(bias_zero=bias_zero)
    shared = _host_inputs(inputs)
    in_maps = [_core_inputs(inputs, shared, c) for c in range(8)]
    res = run_bass_kernel_spmd(nc, in_maps, core_ids=list(range(8)))
    global LAST_RESULT
    LAST_RESULT = res

    cache = np.asarray(inputs["block_cache"], np.float32)
    out = np.empty((B, Q, M), np.float32)
    new_cache = np.empty((B, 2, 2 * KV, H, D), np.float32)
    new_cache[:, :, :KV] = cache
    for c in range(8):
        bi, half = c // 2, c % 2
        r = res.results[c]
        for si, J in enumerate(SLOTS[half]):
            rows = slice(si * P, (si + 1) * P)
            out[bi, J * P : (J + 1) * P] = r["out"][rows]
            new_cache[bi, 0, KV + J * P : KV + (J + 1) * P] = r["k_new"][rows].reshape(
                P, H, D
            )
            new_cache[bi, 1, KV + J * P : KV + (J + 1) * P] = r["v_new"][rows].reshape(
                P, H, D
            )
    return out, new_cache
